# revision 1
# baseline (speedup 1.0000x reference)
"""Trainium2 Bass kernel for nn_NetAtom (Behler-Parrinello segment reduce).

Full-input contract: kernel(**inputs) takes the complete (unsharded) numpy
arrays from setup_inputs() and returns the full [2K] output.

Strategy (8 cores, atom sharding):
  - Host: shard atoms across the 8 cores (padded to 12800/core; padded logic
    rows are zero so padded atoms contribute nothing), pre-transpose desc to
    [D, n] bf16, and pre-pack logic.T into the exact per-partition SBUF
    stream layout [128, n_chunks * 4 * K] fp8 so every logic DMA is a large
    fully-contiguous transfer.
  - Device (per core, bf16 MLP matmuls + fp8 matvec, fp32 PSUM accumulation):
      h1T = tanh(W1 @ descT + b1)          [256, n]  (2 partition tiles)
      h2T = tanh(W2 @ h1T + b2)            [256, n]
      per 128-atom subchunk j:  pv[n,2] = h2T_j.T @ W3T
      v[:,0] = pv[:,0] + b3[0]   (DVE)
      v[:,1] = softplus(pv[:,1] + b3[1]) = Ln(Exp(.) + 1)  (ACT)
      psum[2,500] (x2 banks) += v_j.T @ logicT_j   accumulated over all
      subchunks of both species.
  - The Ln lives in a different ACT function set than Tanh/Exp, and each set
    switch costs a ~1.3us table load; chunks are processed in groups of G=8
    with one Ln per group (logic tiles stay resident until their matvec).
  - 3-stage software pipeline (A: loads+L1, B: L2, C: L3+v) so the PE never
    waits on the same chunk's tanh; the group matvec is spread over
    subsequent slots.
  - Host: sum the 8 per-core [2,1000] partials, concat -> [2000].
"""

import contextlib
from collections import deque

import numpy as np
import ml_dtypes

import concourse.mybir as mybir
import concourse.tile as tile
from concourse import bacc
from concourse.bass_utils import run_bass_kernel_spmd

BF = mybir.dt.bfloat16
F8 = mybir.dt.float8e4
F32 = mybir.dt.float32
ACTF = mybir.ActivationFunctionType

D = 128        # descriptor size
H = 256        # hidden width
N = 100000     # atoms per species (full)
K = 1000       # structures
NCORES = 8
CHUNK = 512    # atoms per pipeline chunk
NJ = CHUNK // 128          # 128-atom subchunks per chunk
NCH = 25                   # chunks per core per species
NA = NCH * CHUNK           # 12800 atoms per core (padded); 8*12800 = 102400
KP = 1024                  # padded K stride (16B-aligned j-step)
KH = K // 2                # structure half (one PSUM bank each)
G = 8          # chunks per Ln group
MV_DRAIN = 1   # matvec chunks emitted per pipeline slot
SC = 2         # chunks per logic/desc superchunk DMA

WCOLS = H + 2 * H + 4      # packed weight cols: w1t | w2t | w3t


def build_nc(repeat=None, mode='full'):
    nc = bacc.Bacc()

    ins = {}
    for s in (0, 1):
        ins[f"logicL{s}"] = nc.dram_tensor(f"logicL{s}", [128, NCH * NJ * KP],
                                           F8, kind="ExternalInput")
        ins[f"descT{s}"] = nc.dram_tensor(f"descT{s}", [D, NA], BF,
                                          kind="ExternalInput")
        ins[f"wpack{s}"] = nc.dram_tensor(f"wpack{s}", [128, WCOLS], BF,
                                          kind="ExternalInput")
        ins[f"bpack{s}"] = nc.dram_tensor(f"bpack{s}", [128, 6], F32,
                                          kind="ExternalInput")
    out_d = nc.dram_tensor("out", [2, K], F32, kind="ExternalOutput")

    with tile.TileContext(nc) as tc:
        with tc.tile_pool(name="consts", bufs=1) as consts, \
             tc.tile_pool(name="descp", bufs=3) as descp, \
             tc.tile_pool(name="logicp", bufs=G + 3) as logicp, \
             tc.tile_pool(name="hp", bufs=4) as hp, \
             tc.tile_pool(name="vp", bufs=3) as vp, \
             tc.tile_pool(name="outp", bufs=1) as outp, \
             tc.tile_pool(name="ps_mlp", bufs=5, space="PSUM") as ps_mlp, \
             tc.tile_pool(name="ps_v", bufs=1, space="PSUM") as ps_v, \
             tc.tile_pool(name="ps_mv", bufs=1, space="PSUM") as ps_mv:

            _stack = contextlib.ExitStack()
            if repeat:
                _stack.enter_context(tc.For_i(0, repeat, 1))

            # ---- constants: one packed weight + bias DMA per species ----
            wp, bp = {}, {}
            for s in (0, 1):
                wp[s] = consts.tile([128, WCOLS], BF, name=f"wp_{s}")
                nc.sync.dma_start(out=wp[s], in_=ins[f"wpack{s}"][:, :])
                bp[s] = consts.tile([128, 6], F32, name=f"bp_{s}")
                nc.sync.dma_start(out=bp[s], in_=ins[f"bpack{s}"][:, :])

            def w1(s, ht):           # [128 d, 128 h]
                return wp[s][:, ht * 128:(ht + 1) * 128]

            def w2(s, kk, ht):       # [128 h1, 128 h2]
                return wp[s][:, H + kk * H + ht * 128:
                             H + kk * H + (ht + 1) * 128]

            def w3(s, kk):           # [128 h2, 2]
                return wp[s][:, 3 * H + 2 * kk:3 * H + 2 * kk + 2]

            def bias(s, which, i):   # [128, 1] per-partition
                off = {"b1": 0, "b2": 2, "b3": 4}[which] + i
                return bp[s][:, off:off + 1]

            # ---- matvec accumulators: [2, 500] x2, live for whole kernel ----
            pmv = [ps_mv.tile([2, KH], F32, name=f"pmv{h}") for h in (0, 1)]

            # chunk descriptors: (species, chunk index within species)
            chunks = [(s, c) for s in (0, 1) for c in range(NCH)]
            n_chunks = len(chunks)
            mv_emitted = [0]
            last_mv = [None]
            super_state = {}

            def stage_a(cdesc):
                """Superchunk DMA loads + layer 1 + tanh(h1)."""
                s, c = cdesc
                if c % SC == 0:
                    nsc = min(SC, NCH - c)
                    dt = descp.tile([D, SC * CHUNK], BF, name="dt", tag="dt")
                    nc.gpsimd.dma_start(
                        out=dt[:, :nsc * CHUNK],
                        in_=ins[f"descT{s}"][:, c * CHUNK:(c + nsc) * CHUNK])
                    lt = logicp.tile([128, SC * NJ, KP], F8, name="lt",
                                     tag="lt")
                    nc.sync.dma_start(
                        out=lt[:, :nsc * NJ, :],
                        in_=ins[f"logicL{s}"][:, c * NJ * KP:(c + nsc) * NJ * KP]
                            .rearrange("p (j k) -> p j k", k=KP),
                    )
                    super_state["lt"] = lt
                    super_state["dt"] = dt
                off = c % SC
                lt = super_state["lt"][:, off * NJ:(off + 1) * NJ, :]
                dtc = super_state["dt"][:, off * CHUNK:(off + 1) * CHUNK]
                if mode == 'dma':
                    return dict(s=s, lt=lt, h1=None)
                h1 = hp.tile([128, 2, CHUNK], BF, name="h1", tag="h1")
                for ht in (0, 1):
                    p1 = ps_mlp.tile([128, CHUNK], F32, name="pmlp",
                                     tag="pmlp")
                    nc.tensor.matmul(
                        p1[:, :], lhsT=w1(s, ht), rhs=dtc,
                        start=True, stop=True,
                    )
                    nc.scalar.activation(
                        h1[:, ht, :], p1[:, :], ACTF.Tanh,
                        bias=bias(s, "b1", ht), scale=1.0,
                    )
                return dict(s=s, lt=lt, h1=h1)

            def stage_b(meta):
                """Layer 2 + tanh(h2)."""
                s, h1 = meta["s"], meta["h1"]
                h2 = hp.tile([128, 2, CHUNK], BF, name="h2", tag="h2")
                for ht in (0, 1):
                    p2 = ps_mlp.tile([128, CHUNK], F32, name="pmlp",
                                     tag="pmlp")
                    for kk in (0, 1):
                        nc.tensor.matmul(
                            p2[:, :], lhsT=w2(s, kk, ht), rhs=h1[:, kk, :],
                            start=(kk == 0), stop=(kk == 1),
                        )
                    nc.scalar.activation(
                        h2[:, ht, :], p2[:, :], ACTF.Tanh,
                        bias=bias(s, "b2", ht), scale=1.0,
                    )
                meta["h2"] = h2

            def stage_c(meta, grp):
                """Layer 3 + v-even (DVE) + exp stash."""
                s, h2 = meta["s"], meta["h2"]
                pv = ps_v.tile([128, 2 * NJ], F32, name="pv", tag="pv")
                for j in range(NJ):
                    for kk in (0, 1):
                        mm = nc.tensor.matmul(
                            pv[:, 2 * j:2 * j + 2],
                            lhsT=h2[:, kk, j * 128:(j + 1) * 128],
                            rhs=w3(s, kk),
                            start=(kk == 0), stop=(kk == 1),
                        )
                        # keep L3 behind this slot's matvec burst in the PE
                        # stream: its tanh(h2) input lands late, and hoisting
                        # it ahead of ready matvec work stalls the PE.
                        if j == 0 and kk == 0 and last_mv[0] is not None:
                            tile.add_dep_helper(
                                mm.ins, last_mv[0].ins, sync=False,
                                reason="order L3 after matvec burst")

                jj = grp["jj"]
                nc.vector.tensor_scalar_add(
                    grp["vg"][:, jj:jj + NJ, 0],
                    pv[:, 0:2 * NJ:2],
                    bias(s, "b3", 0),
                )
                nc.scalar.activation(
                    grp["tg"][:, jj:jj + NJ], pv[:, 1:2 * NJ:2], ACTF.Exp,
                    bias=bias(s, "b3", 1), scale=1.0,
                )
                meta["vg"] = grp["vg"]
                meta["jj"] = jj
                grp["jj"] = jj + NJ

            def emit_ln(grp):
                gnj = grp["jj"]
                nc.scalar.activation(
                    grp["vg"][:, :gnj, 1], grp["tg"][:, :gnj], ACTF.Ln,
                    bias=1.0, scale=1.0,
                )

            def emit_mv(meta):
                if mode == 'nomv':
                    mv_emitted[0] += 1
                    return
                lt, vg, jj = meta["lt"], meta["vg"], meta["jj"]
                first = mv_emitted[0] == 0
                last = mv_emitted[0] == n_chunks - 1
                for jp in range(0, NJ, 2):
                    for h in (0, 1):
                        last_mv[0] = nc.tensor.matmul(
                            pmv[h][:, :],
                            lhsT=vg[:, jj + jp:jj + jp + 2, 0:2],
                            rhs=lt[:, jp:jp + 2, h * KH:(h + 1) * KH],
                            start=(first and jp == 0),
                            stop=(last and jp == NJ - 2),
                            perf_mode=mybir.MatmulPerfMode.DoubleRow,
                            skip_group_check=True,
                        )
                mv_emitted[0] += 1

            def new_grp():
                return dict(
                    vg=vp.tile([128, G * NJ, 16], F8, name="vg", tag="vg"),
                    tg=vp.tile([128, G * NJ], F32, name="tg", tag="tg"),
                    jj=0, metas=[],
                )

            pending = deque()
            prev_a = None
            prev_b = None
            grp = None
            for ci in range(n_chunks + 2):
                meta = stage_a(chunks[ci]) if ci < n_chunks else None
                if mode == 'dma':
                    continue
                if prev_a is not None:
                    stage_b(prev_a)
                for _ in range(MV_DRAIN):
                    if pending:
                        emit_mv(pending.popleft())
                if prev_b is not None:
                    if grp is None:
                        grp = new_grp()
                    stage_c(prev_b, grp)
                    grp["metas"].append(prev_b)
                    if len(grp["metas"]) == G or prev_a is None:
                        emit_ln(grp)
                        pending.extend(grp["metas"])
                        grp = None
                prev_b = prev_a
                prev_a = meta

            while pending:
                emit_mv(pending.popleft())

            # ---- writeback ----
            osb = outp.tile([2, K], F32, name="osb")
            if mode == 'full':
                for h in (0, 1):
                    nc.vector.tensor_copy(osb[:, h * KH:(h + 1) * KH],
                                          pmv[h][:, :])
            else:
                nc.vector.memset(osb[:, :], 0.0)
            nc.sync.dma_start(out=out_d[:, :], in_=osb[:, :])
            _stack.close()

    nc.compile()
    return nc


_NC_CACHE = None


def _get_nc():
    global _NC_CACHE
    if _NC_CACHE is None:
        _NC_CACHE = build_nc()
    return _NC_CACHE


def make_in_maps(desc0, desc1, logic0, logic1,
                 W1_0, b1_0, W2_0, b2_0, W3_0, b3_0,
                 W1_1, b1_1, W2_1, b2_1, W3_1, b3_1):
    bf16 = ml_dtypes.bfloat16
    fp8 = ml_dtypes.float8_e4m3
    NPAD = NCORES * NA

    per_species = {}
    for s, (desc, logic, W1, b1v, W2, b2v, W3, b3v) in enumerate((
            (desc0, logic0, W1_0, b1_0, W2_0, b2_0, W3_0, b3_0),
            (desc1, logic1, W1_1, b1_1, W2_1, b2_1, W3_1, b3_1))):
        descT = np.zeros((D, NPAD), dtype=bf16)
        descT[:, :N] = np.asarray(desc, np.float32).T.astype(bf16)
        logicT = np.zeros((NPAD, KP), dtype=fp8)
        logicT[:N, :K] = np.asarray(logic, np.float32).T.astype(fp8)
        # SBUF stream layout: [core][128, NCH*NJ*KP], chunk c at cols
        # c*NJ*KP, subchunk j contiguous KP cols, partition = atom % 128.
        logicL = (logicT.reshape(NCORES, NCH, NJ, 128, KP)
                  .transpose(0, 3, 1, 2, 4)
                  .reshape(NCORES, 128, NCH * NJ * KP))
        logicL = np.ascontiguousarray(logicL)

        w1t = np.asarray(W1, np.float32).T                   # [128, 256]
        w2t = (np.asarray(W2, np.float32).T.reshape(2, 128, H)
               .transpose(1, 0, 2).reshape(128, 2 * H))      # [128, 512]
        w3t = (np.asarray(W3, np.float32).T.reshape(2, 128, 2)
               .transpose(1, 0, 2).reshape(128, 4))          # [128, 4]
        wpack = np.concatenate([w1t, w2t, w3t], axis=1).astype(bf16)

        bpack = np.concatenate([
            np.asarray(b1v, np.float32).reshape(2, 128).T,
            np.asarray(b2v, np.float32).reshape(2, 128).T,
            np.broadcast_to(np.asarray(b3v, np.float32), (128, 2)),
        ], axis=1)
        bpack = np.ascontiguousarray(bpack)

        per_species[s] = dict(descT=descT, logicL=logicL,
                              wpack=wpack, bpack=bpack)

    in_maps = []
    for c in range(NCORES):
        m = {}
        for s in (0, 1):
            sp = per_species[s]
            m[f"descT{s}"] = sp["descT"][:, c * NA:(c + 1) * NA]
            m[f"logicL{s}"] = sp["logicL"][c]
            m[f"wpack{s}"] = sp["wpack"]
            m[f"bpack{s}"] = sp["bpack"]
        in_maps.append(m)
    return in_maps


def run(in_maps, trace=False, **kwargs):
    nc = _get_nc()
    return run_bass_kernel_spmd(nc, in_maps, core_ids=list(range(NCORES)),
                                trace=trace, **kwargs)


def kernel(**inputs):
    in_maps = make_in_maps(**inputs)
    res = run(in_maps)
    total = np.zeros((2, K), np.float64)
    for r in res.results:
        total += r["out"].astype(np.float64)
    return np.concatenate([total[0], total[1]]).astype(np.float32)



# revision 2
# speedup vs baseline: 1.1443x; 1.1443x over previous
"""Trainium2 Bass kernel for nn_NetAtom (Behler-Parrinello segment reduce), v2.

Full-input contract: kernel(**inputs) takes the complete (unsharded) numpy
arrays from setup_inputs() and returns the full [2K] output.

Architecture (8 cores, atom sharding, all-fp8 datapath):
  - 1024-atom PSUM tiles: each tanh is one [128,1024] ACT instruction
    (amortizes the ~300-cycle per-instruction ACT overhead; ACT is the
    bottleneck engine).
  - softplus(x) computed as (s+|s|)/2 + poly3(exp(-|s|)) using only the
    exp_and_others ACT table set (tanh+exp) -> zero table switches.
  - fp8 everywhere: desc/weights/h1/h2/logic/vg. L2 and L3 use DoubleRow
    (contraction 256 in one pass). L3 keeps W3 stationary (out [2, atoms]),
    then the idle DVE block-transposes [2,1024] -> atom-partition layout;
    the 32x32-block permutation is absorbed into the host-side logic pack.
  - matvec: psum[2,512]+[2,488] accumulated over all tiles with fp8
    DoubleRow matmuls (FD ~500), interleaved as PE filler.
  - PSUM: 4 banks MLP ping-pong, 2 banks L3 out, 2 banks matvec accum.
"""

import contextlib
from collections import deque

import numpy as np
import ml_dtypes

import concourse.mybir as mybir
import concourse.tile as tile
from concourse import bacc
from concourse.bass_utils import run_bass_kernel_spmd

BF = mybir.dt.bfloat16
F8 = mybir.dt.float8e4
F32 = mybir.dt.float32
ACTF = mybir.ActivationFunctionType
ALU = mybir.AluOpType
DR = mybir.MatmulPerfMode.DoubleRow

D = 128        # descriptor size
H = 256        # hidden width
N = 100000     # atoms per species (full)
K = 1000       # structures
KH0, KH1 = 512, 488   # matvec column split (psum bank limit)
NCORES = 8
TILE = 1024            # atoms per psum/ACT tile
NA = 12800             # atoms per core per species (12 full tiles + 1 half)
NFULL = 12
NT = NFULL + 1         # tiles per species (last is 512)
KP = 1024              # padded K stride in logic pack
GRP = 4                # tiles per softplus group
W2OFF = H              # wpack col offsets (fp8)
W3OFF = H + 2 * H
WCOLS = W3OFF + 32

# ln(1+t) ~= t*(P0 + P1*t + P2*t^2) on [0,1], constrained through 0
_t = (1 - np.cos(np.linspace(0, np.pi, 4001))) / 2
_A = np.stack([_t, _t**2, _t**3], 1)
_w = 1.0 / (np.abs(np.log1p(_t)) + 0.05)
P0, P1, P2 = np.linalg.lstsq((_A * _w[:, None]), np.log1p(_t) * _w,
                             rcond=None)[0].tolist()


def tiles_list():
    """[(species, size, tile_idx_in_species)] for the 26 tile units."""
    out = []
    for s in (0, 1):
        for t in range(NT):
            out.append((s, TILE if t < NFULL else TILE // 2, t))
    return out


def build_nc(repeat=None, mode='full'):
    nc = bacc.Bacc()

    ins = {}
    for s in (0, 1):
        ins[f"logicL{s}"] = nc.dram_tensor(f"logicL{s}", [128, NT * 8 * KP],
                                           F8, kind="ExternalInput")
        ins[f"descT{s}"] = nc.dram_tensor(f"descT{s}", [D, NA], F8,
                                          kind="ExternalInput")
        ins[f"wpack{s}"] = nc.dram_tensor(f"wpack{s}", [128, WCOLS], F8,
                                          kind="ExternalInput")
        ins[f"bpack{s}"] = nc.dram_tensor(f"bpack{s}", [128, 6], F32,
                                          kind="ExternalInput")
    out_d = nc.dram_tensor("out", [2, K], F32, kind="ExternalOutput")

    TL = tiles_list()
    n_tiles = len(TL)

    with tile.TileContext(nc) as tc:
        with tc.tile_pool(name="consts", bufs=1) as consts, \
             tc.tile_pool(name="descp", bufs=4) as descp, \
             tc.tile_pool(name="logicp", bufs=10) as logicp, \
             tc.tile_pool(name="hp", bufs=4) as hp, \
             tc.tile_pool(name="vtp", bufs=3) as vtp, \
             tc.tile_pool(name="sgp", bufs=2) as sgp, \
             tc.tile_pool(name="scr", bufs=5) as scr, \
             tc.tile_pool(name="vgp", bufs=3) as vgp, \
             tc.tile_pool(name="outp", bufs=1) as outp, \
             tc.tile_pool(name="ps_mlp", bufs=2, space="PSUM") as ps_mlp, \
             tc.tile_pool(name="ps_pv", bufs=1, space="PSUM") as ps_pv, \
             tc.tile_pool(name="ps_mv", bufs=1, space="PSUM") as ps_mv:

            _stack = contextlib.ExitStack()
            if repeat:
                _stack.enter_context(tc.For_i(0, repeat, 1))

            # ---- constants ----
            wp, bp = {}, {}
            for s in (0, 1):
                wp[s] = consts.tile([128, WCOLS], F8, name=f"wp_{s}")
                nc.sync.dma_start(out=wp[s], in_=ins[f"wpack{s}"][:, :])
                bp[s] = consts.tile([128, 6], F32, name=f"bp_{s}")
                nc.sync.dma_start(out=bp[s], in_=ins[f"bpack{s}"][:, :])

            def w1(s, ht):
                return wp[s][:, ht * 128:(ht + 1) * 128]

            def w2dr(s, ht):
                return wp[s][:, W2OFF + ht * H:W2OFF + (ht + 1) * H] \
                    .rearrange("p (t m) -> p t m", t=2)

            def w3dr(s):
                return wp[s][:, W3OFF:W3OFF + 32] \
                    .rearrange("p (t m) -> p t m", t=2)[:, :, 0:2]

            def bias(s, which, i):
                off = {"b1": 0, "b2": 2, "b3_0": 4, "b3_1h": 5}[which] + i
                return bp[s][:, off:off + 1]

            # ---- standing psum: L3 out + matvec accumulators ----
            pv2 = ps_pv.tile([32, TILE], F32, name="pv2")
            pmv = [ps_mv.tile([2, KH0], F32, name="pmv0"),
                   ps_mv.tile([2, KH1], F32, name="pmv1")]
            nc.vector.memset(pv2[:, :], 0.0)

            mv_state = dict(emitted=[0, 0], total=n_tiles * 4, last=None)
            pending = deque()    # (vg_tile, vg_off, lt, sz)

            def stage_dma(ti):
                s, sz, t = TL[ti]
                dt = descp.tile([D, TILE], F8, name="dt", tag="dt")
                nc.gpsimd.dma_start(
                    out=dt[:, :sz],
                    in_=ins[f"descT{s}"][:, t * TILE:t * TILE + sz])
                lt = logicp.tile([128, 8, KP], F8, name="lt", tag="lt")
                nc.sync.dma_start(
                    out=lt[:, :, :],
                    in_=ins[f"logicL{s}"][:, t * 8 * KP:(t + 1) * 8 * KP]
                        .rearrange("p (j k) -> p j k", k=KP))
                return dict(s=s, sz=sz, dt=dt, lt=lt)

            def stage_l1(meta):
                """L1 matmuls + tanh -> h1 (fp8)."""
                s, sz, dt = meta["s"], meta["sz"], meta["dt"]
                if mode == 'dma':
                    return
                h1 = hp.tile([128, 2, TILE], F8, name="h1", tag="h")
                for ht in (0, 1):
                    p = ps_mlp.tile([128, TILE], F32, name="pm", tag="pm")
                    for c in range(0, sz, 512):
                        nc.tensor.matmul(
                            p[:, c:c + 512], lhsT=w1(s, ht),
                            rhs=dt[:, c:c + 512], start=True, stop=True)
                    if mode != 'noact':
                        nc.scalar.activation(
                            h1[:, ht, :sz], p[:, :sz], ACTF.Tanh,
                            bias=bias(s, "b1", ht), scale=1.0)
                    else:
                        nc.vector.memset(h1[:, ht, :sz], 0.25)
                meta["h1"] = h1

            def stage_l2mm(meta):
                """L2 DoubleRow matmuls (tanh2 emitted a slot later)."""
                s, sz, h1 = meta["s"], meta["sz"], meta["h1"]
                meta["h2"] = hp.tile([128, 2, TILE], F8, name="h2", tag="h")
                meta["p2"] = []
                for ht in (0, 1):
                    p = ps_mlp.tile([128, TILE], F32, name="pm", tag="pm")
                    for c in range(0, sz, 512):
                        nc.tensor.matmul(
                            p[:, c:c + 512], lhsT=w2dr(s, ht),
                            rhs=h1[:, :, c:c + 512], start=True, stop=True,
                            perf_mode=DR)
                    meta["p2"].append(p)

            def stage_tanh2(meta):
                s, sz, h2 = meta["s"], meta["sz"], meta["h2"]
                for ht in (0, 1):
                    p = meta["p2"][ht]
                    if mode != 'noact':
                        nc.scalar.activation(
                            h2[:, ht, :sz], p[:, :sz], ACTF.Tanh,
                            bias=bias(s, "b2", ht), scale=1.0)
                    else:
                        nc.vector.memset(h2[:, ht, :sz], 0.25)

            def stage_l3(meta, grp):
                """L3 DoubleRow (W3 stationary) -> pv2[2, sz] -> DVE block
                transpose -> vt [128, 256] -> per-tile evac into sg/vg."""
                s, sz, h2 = meta["s"], meta["sz"], meta["h2"]
                for c in range(0, sz, 512):
                    mm = nc.tensor.matmul(
                        pv2[0:2, c:c + 512], lhsT=w3dr(s),
                        rhs=h2[:, :, c:c + 512], start=True, stop=True,
                        perf_mode=DR, skip_group_check=True)
                    if mv_state["last"] is not None:
                        tile.add_dep_helper(
                            mm.ins, mv_state["last"].ins, sync=False,
                            reason="order L3 after matvec burst")
                vt = vtp.tile([128, 256], F32, name="vt", tag="vt")
                if sz < TILE:
                    nc.vector.memset(vt[:, :], 0.0)
                for g in range(sz // 256):
                    nc.vector.transpose(
                        out=vt[32 * g:32 * (g + 1), :],
                        in_=pv2[:, 256 * g:256 * (g + 1)])
                q = grp["q"]
                # v0 = vt[:, 0::32] + b3[0]  (fp8 into vg)
                nc.vector.tensor_scalar(
                    grp["vg"][:, 8 * q:8 * q + 8, 0],
                    vt[:, 0:256:32], bias(s, "b3_0", 0), None, op0=ALU.add)
                # s2 = 0.5*vt[:, 1::32] + 0.5*b3[1]
                nc.vector.tensor_scalar(
                    grp["sg"][:, 8 * q:8 * q + 8],
                    vt[:, 1:256:32], 0.5, bias(s, "b3_1h", 0),
                    op0=ALU.mult, op1=ALU.add)
                grp["q"] = q + 1
                grp["tiles"].append((meta["lt"], 8 * q, meta["sz"]))

            def finish_group(grp):
                """softplus tail: |s|, exp, poly, vg[...,1]."""
                ncols = 8 * grp["q"]
                sg = grp["sg"][:, :ncols]
                pp = scr.tile([128, 8 * GRP], F32, name="pp", tag="pp",
                              bufs=2)[:, :ncols]
                nc.vector.tensor_scalar(pp, sg, 0.0, None, op0=ALU.max)
                mm_ = scr.tile([128, 8 * GRP], F32, name="mm", tag="scr")[:, :ncols]
                nc.vector.tensor_scalar(mm_, sg, -1.0, 0.0,
                                        op0=ALU.mult, op1=ALU.max)
                ag = scr.tile([128, 8 * GRP], F32, name="ag", tag="scr")[:, :ncols]
                nc.vector.tensor_tensor(ag, pp, mm_, op=ALU.add)
                tg = scr.tile([128, 8 * GRP], F32, name="tg", tag="scr")[:, :ncols]
                nc.scalar.activation(tg, ag, ACTF.Exp, bias=0.0, scale=-2.0)
                pa = scr.tile([128, 8 * GRP], F32, name="pa", tag="scr")[:, :ncols]
                nc.vector.tensor_scalar(pa, tg, P2, P1, op0=ALU.mult, op1=ALU.add)
                pb = scr.tile([128, 8 * GRP], F32, name="pb", tag="scr")[:, :ncols]
                nc.vector.tensor_tensor(pb, pa, tg, op=ALU.mult)
                nc.vector.tensor_scalar(pa, pb, P0, None, op0=ALU.add)
                nc.vector.tensor_tensor(pb, pa, tg, op=ALU.mult)
                # vg1 = (s2 + |s2|) + poly = 2*relu(s2) + poly
                nc.vector.tensor_scalar(pa, pp, 2.0, None, op0=ALU.mult)
                nc.vector.tensor_tensor(
                    grp["vg"][:, :grp["q"] * 8, 1], pa, pb, op=ALU.add)
                for item in grp["tiles"]:
                    pending.append((grp["vg"], *item))

            def emit_mv(n):
                for _ in range(n):
                    if not pending:
                        return
                    vg, lt, off, sz = pending.popleft()
                    if mode == 'nomv':
                        mv_state["emitted"][0] += 4
                        mv_state["emitted"][1] += 4
                        continue
                    for jp in range(0, 8, 2):
                        for h, (k0, kw) in enumerate(((0, KH0), (KH0, KH1))):
                            first = mv_state["emitted"][h] == 0
                            last = mv_state["emitted"][h] == mv_state["total"] - 1
                            mv_state["last"] = nc.tensor.matmul(
                                pmv[h][:, :],
                                lhsT=vg[:, off + jp:off + jp + 2, 0:2],
                                rhs=lt[:, jp:jp + 2, k0:k0 + kw],
                                start=first, stop=last,
                                perf_mode=DR, skip_group_check=True)
                            mv_state["emitted"][h] += 1

            def new_grp():
                return dict(
                    vg=vgp.tile([128, 8 * GRP, 16], F8, name="vg", tag="vg"),
                    sg=sgp.tile([128, 8 * GRP], F32, name="sg", tag="sg"),
                    q=0, tiles=[])

            # Slot i: ACT runs [tanh2(i-2) x2, tanh1(i) x2] -- every input
            # was produced in an earlier slot, so ACT never waits mid-chain.
            # PE runs [MV, L1(i), MV, L3(i-2), L2mm(i-1)].
            PRE = 2
            metas = {}
            grp = None
            done_grp = None
            for i in range(PRE):
                metas[i] = stage_dma(i)
            for i in range(n_tiles + 2):
                # prefetch DMA
                if i + PRE < n_tiles:
                    metas[i + PRE] = stage_dma(i + PRE)
                if mode == 'dma':
                    continue
                emit_mv(1)
                # tanh2 for tile i-2 (L2 matmuls ran at the end of slot i-1)
                if 0 <= i - 2 < n_tiles:
                    stage_tanh2(metas[i - 2])
                # L1 + tanh1 for tile i
                if i < n_tiles:
                    stage_l1(metas[i])
                emit_mv(1)
                # L3 + transpose + evac for tile i-2 (after its tanh2)
                if 0 <= i - 2 < n_tiles:
                    if grp is None:
                        grp = new_grp()
                    stage_l3(metas[i - 2], grp)
                    if grp["q"] == GRP or i - 2 == n_tiles - 1:
                        done_grp, grp = grp, None
                # L2 matmuls for tile i-1 (deps from slot i-1: ancient)
                if 0 <= i - 1 < n_tiles:
                    stage_l2mm(metas[i - 1])
                # group tail
                if done_grp is not None and mode != 'noact':
                    finish_group(done_grp)
                    done_grp = None
                elif done_grp is not None:
                    for item in done_grp["tiles"]:
                        pending.append((done_grp["vg"], *item))
                    done_grp = None
                if 0 <= i - 2 < n_tiles:
                    del metas[i - 2]

            while pending:
                emit_mv(len(pending))

            # ---- writeback ----
            osb = outp.tile([2, K], F32, name="osb")
            if mode in ('full', 'noact'):
                nc.vector.tensor_copy(osb[:, 0:KH0], pmv[0][:, :])
                nc.vector.tensor_copy(osb[:, KH0:K], pmv[1][:, :])
            else:
                nc.vector.memset(osb[:, :], 0.0)
            nc.sync.dma_start(out=out_d[:, :], in_=osb[:, :])
            _stack.close()

    nc.compile()
    return nc


_NC_CACHE = None


def _get_nc():
    global _NC_CACHE
    if _NC_CACHE is None:
        _NC_CACHE = build_nc()
    return _NC_CACHE


def _pack_logic(logic):
    """[K, N] -> per-core [128, NT*8*KP] fp8 with the transpose-block
    atom mapping: atom a (in tile) -> partition 32*(a//256)+(a%32),
    j-slot (a//32)%8."""
    fp8 = ml_dtypes.float8_e4m3
    NPAD = NCORES * NA
    logicT = np.zeros((NPAD, KP), dtype=fp8)
    logicT[:N, :K] = np.asarray(logic, np.float32).T.astype(fp8)
    per_core = logicT.reshape(NCORES, NA, KP)
    full = per_core[:, :NFULL * TILE].reshape(NCORES, NFULL, 4, 8, 32, KP)
    full = full.transpose(0, 1, 3, 2, 4, 5)   # [c, t, j, g, r, KP]
    # want [c, t, P=(g,r), j, KP]
    full = full.transpose(0, 1, 3, 4, 2, 5)   # [c, t, g, r, j, KP]
    full = full.reshape(NCORES, NFULL, 128, 8, KP)
    half = per_core[:, NFULL * TILE:].reshape(NCORES, 2, 8, 32, KP)
    half = half.transpose(0, 1, 3, 2, 4)      # [c, g, r, j, KP]
    half = half.reshape(NCORES, 64, 8, KP)
    halfp = np.zeros((NCORES, 1, 128, 8, KP), dtype=fp8)
    halfp[:, 0, :64] = half
    allt = np.concatenate([full, halfp], axis=1)  # [c, NT, 128, 8, KP]
    out = np.ascontiguousarray(allt.transpose(0, 2, 1, 3, 4)
                               .reshape(NCORES, 128, NT * 8 * KP))
    return out


def make_in_maps(desc0, desc1, logic0, logic1,
                 W1_0, b1_0, W2_0, b2_0, W3_0, b3_0,
                 W1_1, b1_1, W2_1, b2_1, W3_1, b3_1):
    fp8 = ml_dtypes.float8_e4m3
    NPAD = NCORES * NA

    per_species = {}
    for s, (desc, logic, W1, b1v, W2, b2v, W3, b3v) in enumerate((
            (desc0, logic0, W1_0, b1_0, W2_0, b2_0, W3_0, b3_0),
            (desc1, logic1, W1_1, b1_1, W2_1, b2_1, W3_1, b3_1))):
        descT = np.zeros((D, NPAD), dtype=fp8)
        descT[:, :N] = np.asarray(desc, np.float32).T.astype(fp8)
        logicL = _pack_logic(logic)

        w1t = np.asarray(W1, np.float32).T                    # [128, 256]
        W2f = np.asarray(W2, np.float32)
        # w2dr[p, ht*256 + kk*128 + m] = W2[ht*128+m, kk*128+p]
        w2t = (W2f.reshape(2, 128, 2, 128).transpose(3, 0, 2, 1)
               .reshape(128, 2 * H))
        W3f = np.asarray(W3, np.float32)                      # [2, 256]
        w3t = np.zeros((128, 32), dtype=np.float32)
        for kk in range(2):
            for m in range(2):
                w3t[:, kk * 16 + m] = W3f[m, kk * 128:(kk + 1) * 128]
        wpack = np.concatenate([w1t, w2t, w3t], axis=1).astype(fp8)

        b3f = np.asarray(b3v, np.float32)
        bpack = np.concatenate([
            np.asarray(b1v, np.float32).reshape(2, 128).T,
            np.asarray(b2v, np.float32).reshape(2, 128).T,
            np.full((128, 1), b3f[0], np.float32),
            np.full((128, 1), 0.5 * b3f[1], np.float32),
        ], axis=1)
        per_species[s] = dict(descT=descT, logicL=logicL,
                              wpack=np.ascontiguousarray(wpack),
                              bpack=np.ascontiguousarray(bpack))

    in_maps = []
    for c in range(NCORES):
        m = {}
        for s in (0, 1):
            sp = per_species[s]
            m[f"descT{s}"] = sp["descT"][:, c * NA:(c + 1) * NA]
            m[f"logicL{s}"] = sp["logicL"][c]
            m[f"wpack{s}"] = sp["wpack"]
            m[f"bpack{s}"] = sp["bpack"]
        in_maps.append(m)
    return in_maps


def run(in_maps, trace=False, **kwargs):
    nc = _get_nc()
    return run_bass_kernel_spmd(nc, in_maps, core_ids=list(range(NCORES)),
                                trace=trace, **kwargs)


def kernel(**inputs):
    in_maps = make_in_maps(**inputs)
    res = run(in_maps)
    total = np.zeros((2, K), np.float64)
    for r in res.results:
        total += r["out"].astype(np.float64)
    return np.concatenate([total[0], total[1]]).astype(np.float32)
